# revision 3
# baseline (speedup 1.0000x reference)
"""Trainium2 kernel for nn_LinearRowShared4Bit: out = x @ W.T + bias where W is
dequantized from 4-bit packed weights with per-(16-row-group, 32-col-block) norms.

Strategy (8-core tensor-parallel over out_features, 1024 rows each):
  - View int32 packed weights (value = byte B in low 8 bits) as uint16 pairs
    [B, 0]; DMA-transpose 128-column slices into SBUF -> tiles T [128p, 1024o]
    with B on even partitions, 0 on odd. Partition p=2m of chunk c holds byte
    k=64c+m -> (b,h)=(k//16, k%16) -> covers W.T rows i_lo=32b+2h (low nibble)
    and i_hi=i_lo+1 (high nibble).
  - Dequant via 3 DVE ops/tile: L = T & 15; WL = L * s; WB = T * s, where
    s[p, o] = (2/15)*norm[og(o), b(p)] read via a broadcast (0-step) AP.
    Using T = 16H + L:  sum_i x*s*q = sum(xe - xo/16)*WL + sum(xo/16)*WB,
    folded into host-prepped x operands. The "-norm" part of the dequant and
    the bias are applied by a small fp32 side-matmul at the end.
  - PE: x-side stationary [128,16], weights stream as rhs; PSUM [16,1024] fp32
    accumulates over all 64 chunks.
Host gathers per-core [16,1024] outputs -> [16,8192].
"""

import numpy as np

O, I = 8192, 8192
GROUP, SHARE = 32, 16
NCORES = 8
OS = O // NCORES          # 1024 out rows per core
OGS = OS // SHARE         # 64 row-groups per core
NCHUNK = I // 128         # 64 col-chunks of 128 uint16 columns
T_BATCH = 16

_cache = {}


def _build_program():
    import concourse.mybir as mybir
    from concourse import bacc
    from concourse.tile import TileContext

    f16, f32, u16 = mybir.dt.float16, mybir.dt.float32, mybir.dt.uint16
    nc = bacc.Bacc("TRN2", target_bir_lowering=False, debug=False)

    wq16 = nc.dram_tensor("wq16", [OS, I], u16, kind="ExternalInput")
    xe_d = nc.dram_tensor("xe", [128, NCHUNK, T_BATCH], f16, kind="ExternalInput")
    xo_d = nc.dram_tensor("xo", [128, NCHUNK, T_BATCH], f16, kind="ExternalInput")
    sn_d = nc.dram_tensor("snorm", [128, NCHUNK, OGS], f16, kind="ExternalInput")
    xsT_d = nc.dram_tensor("xsT", [128, 2, T_BATCH], f32, kind="ExternalInput")
    nmT_d = nc.dram_tensor("normT", [128, 2, OGS], f32, kind="ExternalInput")
    fx_d = nc.dram_tensor("rhs_fix", [OGS + 1, OS], f32, kind="ExternalInput")
    out_d = nc.dram_tensor("out", [T_BATCH, OS], f32, kind="ExternalOutput")

    with TileContext(nc) as tc:
        with (
            tc.tile_pool(name="const", bufs=1) as const,
            tc.tile_pool(name="tp", bufs=6) as tp,
            tc.tile_pool(name="wp", bufs=4) as wp,
            tc.tile_pool(name="ps", bufs=1, space="PSUM") as ps,
        ):
            # constants / small inputs
            xe_sb = const.tile([128, NCHUNK, T_BATCH], f16)
            nc.sync.dma_start(xe_sb[:], xe_d[:])
            xo_sb = const.tile([128, NCHUNK, T_BATCH], f16)
            nc.sync.dma_start(xo_sb[:], xo_d[:])
            sn_sb = const.tile([128, NCHUNK, OGS], f16)
            nc.sync.dma_start(sn_sb[:], sn_d[:])
            xsT_sb = const.tile([128, 2, T_BATCH], f32)
            nc.sync.dma_start(xsT_sb[:], xsT_d[:])
            nmT_sb = const.tile([128, 2, OGS], f32)
            nc.sync.dma_start(nmT_sb[:], nmT_d[:])
            fx_sb = const.tile([OGS + 1, OS], f32)
            nc.sync.dma_start(fx_sb[:], fx_d[:])
            mask = const.tile([128, 1], u16)
            nc.any.memset(mask, 15)

            psA = ps.tile([T_BATCH, 512], f32)
            psB = ps.tile([T_BATCH, 512], f32)

            # fix path: N.T[og, t] = sum_b norm[og, b] * xs[t, b]
            ps2 = ps.tile([OGS, T_BATCH], f32)
            nc.tensor.matmul(ps2[:], nmT_sb[:, 0, :], xsT_sb[:, 0, :],
                             start=True, stop=False)
            nc.tensor.matmul(ps2[:], nmT_sb[:, 1, :], xsT_sb[:, 1, :],
                             start=False, stop=True)
            fixw = const.tile([OGS + 1, T_BATCH], f32)
            nc.vector.tensor_scalar_mul(fixw[0:OGS, :], ps2[:], -1.0)
            nc.any.memset(fixw[OGS:OGS + 1, :], 1.0)

            # main loop over 64 column chunks
            for c in range(NCHUNK):
                T = tp.tile([128, 1024], u16, tag="T")
                dma = nc.sync if c % 2 == 0 else nc.scalar
                dma.dma_start_transpose(T[:], wq16[:, 128 * c:128 * (c + 1)])

                s_b = sn_sb[:, c, :].unsqueeze(2).broadcast_to([128, OGS, SHARE])
                T3 = T[:].rearrange("p (a b) -> p a b", b=SHARE)

                L = wp.tile([128, 1024], u16, tag="L")
                nc.vector.tensor_scalar(L[:], T[:], mask[:], None,
                                        mybir.AluOpType.bitwise_and)
                WB = wp.tile([128, 1024], f16, tag="WB")
                nc.vector.tensor_tensor(
                    WB[:].rearrange("p (a b) -> p a b", b=SHARE), T3, s_b,
                    mybir.AluOpType.mult)
                WL = wp.tile([128, 1024], f16, tag="WL")
                nc.vector.tensor_tensor(
                    WL[:].rearrange("p (a b) -> p a b", b=SHARE),
                    L[:].rearrange("p (a b) -> p a b", b=SHARE), s_b,
                    mybir.AluOpType.mult)

                nc.tensor.matmul(psA[:], xe_sb[:, c, :], WL[:, 0:512],
                                 start=(c == 0), stop=False)
                nc.tensor.matmul(psB[:], xe_sb[:, c, :], WL[:, 512:1024],
                                 start=(c == 0), stop=False)
                nc.tensor.matmul(psA[:], xo_sb[:, c, :], WB[:, 0:512],
                                 start=False, stop=False)
                nc.tensor.matmul(psB[:], xo_sb[:, c, :], WB[:, 512:1024],
                                 start=False, stop=False)

            # -norm term + bias, then PSUM -> SBUF -> DRAM
            nc.tensor.matmul(psA[:], fixw[:], fx_sb[:, 0:512],
                             start=False, stop=True)
            nc.tensor.matmul(psB[:], fixw[:], fx_sb[:, 512:1024],
                             start=False, stop=True)
            out_sb = const.tile([T_BATCH, OS], f32)
            nc.vector.tensor_copy(out_sb[:, 0:512], psA[:])
            nc.vector.tensor_copy(out_sb[:, 512:1024], psB[:])
            nc.sync.dma_start(out_d[:], out_sb[:])

    nc.finalize()
    return nc


def _prep_shared(x):
    """x-derived operands, identical on every core."""
    xf = x.astype(np.float32)
    k = np.arange(I // 2)                   # int32/byte index
    b, h = k // 16, k % 16
    i_lo = 32 * b + 2 * h
    i_hi = i_lo + 1
    xe_mod = xf[:, i_lo] - xf[:, i_hi] / 16.0   # [16, 4096]
    xo16 = xf[:, i_hi] / 16.0                    # [16, 4096]

    def lanes(a):  # [16, 4096] -> [128, 64, 16] fp16, payload on even lanes
        out = np.zeros((128, NCHUNK, T_BATCH), np.float16)
        # payload lane p=2m of chunk c holds byte k=64c+m
        out[0::2] = a.T.reshape(NCHUNK, 64, T_BATCH).transpose(1, 0, 2)
        return out

    xe = lanes(xe_mod)
    xo = lanes(xo16)

    xs = xf.reshape(T_BATCH, I // GROUP, GROUP).sum(-1)   # [16, 256]
    xsT = np.ascontiguousarray(
        xs.T.reshape(2, 128, T_BATCH).transpose(1, 0, 2)).astype(np.float32)
    return xe, xo, xsT


def kernel(x, weight_q4, weight_norm, bias, _trace=False, _trace_kwargs=None):
    from concourse.bass_utils import run_bass_kernel_spmd

    if "nc" not in _cache:
        _cache["nc"] = _build_program()
    nc = _cache["nc"]

    xe, xo, xsT = _prep_shared(x)
    sel = (np.arange(OS) // SHARE == np.arange(OGS)[:, None]).astype(np.float32)

    in_maps = []
    for m in range(NCORES):
        wq = np.ascontiguousarray(weight_q4[m * OS:(m + 1) * OS]).astype('<i4')
        wq16 = wq.view('<u2').reshape(OS, I)

        norm = weight_norm[m * OGS:(m + 1) * OGS, :, 0].astype(np.float32)  # [64, 256]
        sn = (2.0 / 15.0) * norm
        # snorm_lane[p, c, og] = sn[og, 4c + p//32]
        blk = (4 * np.arange(NCHUNK)[None, :] + (np.arange(128) // 32)[:, None])
        sn_lane = sn.T[blk].astype(np.float16)            # [128, 64, 64]

        nmT = np.ascontiguousarray(
            norm.T.reshape(2, 128, OGS).transpose(1, 0, 2)).astype(np.float32)

        rhs_fix = np.empty((OGS + 1, OS), np.float32)
        rhs_fix[0:OGS] = sel
        rhs_fix[OGS] = bias[m * OS:(m + 1) * OS].astype(np.float32)

        in_maps.append(dict(
            wq16=wq16, xe=xe.view(np.uint16), xo=xo.view(np.uint16),
            snorm=sn_lane.view(np.uint16), xsT=xsT, normT=nmT,
            rhs_fix=rhs_fix))

    res = run_bass_kernel_spmd(nc, in_maps, core_ids=list(range(NCORES)),
                               trace=_trace, **(_trace_kwargs or {}))
    outs = [r["out"] for r in res.results]
    full = np.concatenate(outs, axis=1).astype(np.float32)
    if _trace:
        return full, res
    return full


# revision 12
# speedup vs baseline: 1.5654x; 1.5654x over previous
"""Trainium2 kernel for nn_LinearRowShared4Bit: out = x @ W.T + bias where W is
dequantized from 4-bit packed weights with per-(16-row-group, 32-col-block)
fp16 norms.

8-core tensor-parallel over out_features (1024 rows/core). Per core:

  - View int32 packed weights (value = byte B in low 8 bits) as uint16 pairs
    [B, 0]; DMA-transpose 128-column slices into SBUF -> T tiles [128p, o]
    with B on even partitions, 0 on odd. Partition p=2m of chunk c holds byte
    k=64c+m -> (b,h)=(k//16,k%16) -> W.T rows i_lo=32b+2h (low nib), i_lo+1
    (high nib).
  - Bit-assemble fp16 weights with pure-bitwise DVE ops (no int->fp convert):
      F_L = (T & 15) | 0x6400   == fp16(1024 + L)
      F_T =  T       | 0x6400   == fp16(1024 + T),  T = 16H + L
  - Stage 1 (PE): per chunk-pair, accumulate per-block-slot partials into
    PSUM [128=(16t x 8slot), 1024o]:  ps[16j+t, o] += sum_p xepat*F_L
    + xopat*F_T, where xepat/xopat are host-prepped block-masked x patterns
    (xe - xo/16 and xo/16) so that L/H separate algebraically:
      sum_i x*s*q = sum (xe - xo/16)*(s*L) + sum (xo/16)*(s*T).
  - Stage 2: one fused DVE op per pair: (ps - K[m]) * s_bcast, where K[m] is
    the host-computed 1024-offset contribution, s[m, og(o)] the norm scale;
    then a selector matmul reduces the 8 block-slots into PSUM out [16, 1024].
  - The "-norm" dequant term and bias ride a small fp32 side-matmul.

Host gathers per-core [16, 1024] outputs -> [16, 8192].
"""

import numpy as np

O, I = 8192, 8192
GROUP, SHARE = 32, 16
NCORES = 8
OS = O // NCORES          # 1024 out rows per core
OGS = OS // SHARE         # 64 row-groups per core
NCHUNK = I // 128         # 64 col-chunks of 128 uint16 columns
NPAIR = NCHUNK // 2
T_BATCH = 16

_cache = {}


def _build_program():
    import concourse.mybir as mybir
    from concourse import bacc
    from concourse.tile import TileContext

    f16, f32, u16 = mybir.dt.float16, mybir.dt.float32, mybir.dt.uint16
    alu = mybir.AluOpType
    nc = bacc.Bacc("TRN2", target_bir_lowering=False, debug=False)

    wq16 = nc.dram_tensor("wq16", [OS, I], u16, kind="ExternalInput")
    xep_d = nc.dram_tensor("xepat", [128, NCHUNK, 64], f16, kind="ExternalInput")
    xop_d = nc.dram_tensor("xopat", [128, NCHUNK, 64], f16, kind="ExternalInput")
    s2_d = nc.dram_tensor("s2", [128, NPAIR, OGS], f16, kind="ExternalInput")
    k_d = nc.dram_tensor("koff", [128, NPAIR], f32, kind="ExternalInput")
    sel_d = nc.dram_tensor("sel", [128, T_BATCH], f16, kind="ExternalInput")
    xsT_d = nc.dram_tensor("xsT", [128, 2, T_BATCH], f32, kind="ExternalInput")
    nmT_d = nc.dram_tensor("normT", [128, 2, OGS], f32, kind="ExternalInput")
    fx_d = nc.dram_tensor("rhs_fix", [OGS + 1, OS], f32, kind="ExternalInput")
    out_d = nc.dram_tensor("out", [T_BATCH, OS], f32, kind="ExternalOutput")

    with TileContext(nc) as tc:
        with (
            tc.tile_pool(name="const", bufs=1) as const,
            tc.tile_pool(name="tp", bufs=6) as tp,
            tc.tile_pool(name="wp", bufs=3) as wp,
            tc.tile_pool(name="ps", bufs=1, space="PSUM") as ps,
        ):
            xep_sb = const.tile([128, NCHUNK, 64], f16)
            nc.sync.dma_start(xep_sb[:], xep_d[:])
            xop_sb = const.tile([128, NCHUNK, 64], f16)
            nc.sync.dma_start(xop_sb[:], xop_d[:])
            s2_sb = const.tile([128, NPAIR, OGS], f16)
            nc.sync.dma_start(s2_sb[:], s2_d[:])
            k_sb = const.tile([128, NPAIR], f32)
            nc.sync.dma_start(k_sb[:], k_d[:])
            sel_sb = const.tile([128, T_BATCH], f16)
            nc.sync.dma_start(sel_sb[:], sel_d[:])
            xsT_sb = const.tile([128, 2, T_BATCH], f32)
            nc.sync.dma_start(xsT_sb[:], xsT_d[:])
            nmT_sb = const.tile([128, 2, OGS], f32)
            nc.sync.dma_start(nmT_sb[:], nmT_d[:])
            fx_sb = const.tile([OGS + 1, OS], f32)
            nc.sync.dma_start(fx_sb[:], fx_d[:])
            mask = const.tile([128, 1], u16)
            nc.vector.memset(mask[:], 15)
            orc = const.tile([128, 1], u16)
            nc.vector.memset(orc[:], 0x6400)

            psA = ps.tile([T_BATCH, 512], f32)
            psB = ps.tile([T_BATCH, 512], f32)

            # fix path: N.T[og, t] = sum_b norm[og, b] * xs[t, b]
            ps2 = ps.tile([OGS, T_BATCH], f32, tag="pp", bufs=3)
            nc.tensor.matmul(ps2[:], nmT_sb[:, 0, :], xsT_sb[:, 0, :],
                             start=True, stop=False)
            nc.tensor.matmul(ps2[:], nmT_sb[:, 1, :], xsT_sb[:, 1, :],
                             start=False, stop=True)
            fixw = const.tile([OGS + 1, T_BATCH], f32)
            nc.vector.tensor_scalar_mul(fixw[0:OGS, :], ps2[:], -1.0)
            nc.vector.memset(fixw[OGS:OGS + 1, :], 1.0)

            for pr in range(NPAIR):
                T2 = tp.tile([128, 2048], u16, tag="T")
                d0 = d1 = nc.sync
                d0.dma_start_transpose(T2[:, 0:1024],
                                       wq16[:, 256 * pr:256 * pr + 128])
                d1.dma_start_transpose(T2[:, 1024:2048],
                                       wq16[:, 256 * pr + 128:256 * pr + 256])

                FL = wp.tile([128, 2048], u16, tag="FL", bufs=6)
                nc.vector.tensor_scalar(FL[:], T2[:], mask[:], orc[:],
                                        alu.bitwise_and, alu.bitwise_or)
                FT = wp.tile([128, 2048], u16, tag="FT", bufs=6)
                nc.vector.tensor_scalar(FT[:], T2[:], orc[:], None,
                                        alu.bitwise_or)
                FLh = FL[:].bitcast(f16)
                FTh = FT[:].bitcast(f16)

                pp = ps.tile([128, 1024], f32, tag="pp", bufs=3)
                for h in (0, 1):
                    c = 2 * pr + h
                    xe_l = xep_sb[:, c, :]
                    xo_l = xop_sb[:, c, :]
                    o0 = 1024 * h
                    rows = pp[64 * h:64 * h + 64, :]
                    nc.tensor.matmul(rows[:, 0:512], xe_l,
                                     FLh[:, o0:o0 + 512],
                                     start=True, stop=False)
                    nc.tensor.matmul(rows[:, 512:1024], xe_l,
                                     FLh[:, o0 + 512:o0 + 1024],
                                     start=True, stop=False)
                    nc.tensor.matmul(rows[:, 0:512], xo_l,
                                     FTh[:, o0:o0 + 512],
                                     start=False, stop=True)
                    nc.tensor.matmul(rows[:, 512:1024], xo_l,
                                     FTh[:, o0 + 512:o0 + 1024],
                                     start=False, stop=True)

                sc = wp.tile([128, 1024], f16, tag="SC", bufs=6)
                nc.vector.scalar_tensor_tensor(
                    sc[:].rearrange("p (a b) -> p a b", b=SHARE),
                    pp[:].rearrange("p (a b) -> p a b", b=SHARE),
                    k_sb[:, pr:pr + 1],
                    s2_sb[:, pr, :].unsqueeze(2).broadcast_to([128, OGS, SHARE]),
                    alu.subtract, alu.mult)

                nc.tensor.matmul(psA[:], sel_sb[:], sc[:, 0:512],
                                 start=(pr == 0), stop=False)
                nc.tensor.matmul(psB[:], sel_sb[:], sc[:, 512:1024],
                                 start=(pr == 0), stop=False)

            nc.tensor.matmul(psA[:], fixw[:], fx_sb[:, 0:512],
                             start=False, stop=True)
            nc.tensor.matmul(psB[:], fixw[:], fx_sb[:, 512:1024],
                             start=False, stop=True)
            out_sb = const.tile([T_BATCH, OS], f32)
            nc.vector.tensor_copy(out_sb[:, 0:512], psA[:])
            nc.vector.tensor_copy(out_sb[:, 512:1024], psB[:])
            nc.sync.dma_start(out_d[:], out_sb[:])

    nc.finalize()
    return nc


def _prep_shared(x):
    """x-derived operands, identical on every core."""
    xf = x.astype(np.float64)
    k = np.arange(I // 2)                   # byte index within a row
    b, h = k // 16, k % 16
    i_lo = 32 * b + 2 * h
    xe_mod = xf[:, i_lo] - xf[:, i_lo + 1] / 16.0   # [16, 4096]
    xo16 = xf[:, i_lo + 1] / 16.0                    # [16, 4096]

    def pat(a):
        """[16, 4096] -> [128, 64, 64] fp16 block-slot pattern.

        Payload lane p=2m of chunk c holds byte k=64c+m; its x value goes to
        column m' = 16*(p//32) + t. Odd lanes and other columns stay 0."""
        lane = np.zeros((128, NCHUNK, T_BATCH), np.float16)
        lane[0::2] = a.T.reshape(NCHUNK, 64, T_BATCH).transpose(1, 0, 2)
        out = np.zeros((128, NCHUNK, 64), np.float16)
        for jj in range(4):
            rows = slice(32 * jj, 32 * jj + 32)
            out[rows, :, 16 * jj:16 * jj + 16] = lane[rows]
        return out

    xep = pat(xe_mod)
    xop = pat(xo16)

    # K[m, pr] = 1024 * sum_p (xep + xop)[p, c, m%64] with c = 2pr + m//64,
    # computed from the fp16-rounded patterns (must match device exactly).
    colsum = (xep.astype(np.float64) + xop.astype(np.float64)).sum(axis=0)  # [64c, 64m]
    K = np.zeros((128, NPAIR), np.float32)
    K[0:64] = 1024.0 * colsum[0::2].T
    K[64:128] = 1024.0 * colsum[1::2].T

    sel = (np.arange(128)[:, None] % 16 == np.arange(T_BATCH)[None, :]
           ).astype(np.float16)

    xs = xf.reshape(T_BATCH, I // GROUP, GROUP).sum(-1)   # [16, 256]
    xsT = np.ascontiguousarray(
        xs.T.reshape(2, 128, T_BATCH).transpose(1, 0, 2)).astype(np.float32)
    return xep, xop, K, sel, xsT


def kernel(x, weight_q4, weight_norm, bias, _trace=False, _trace_kwargs=None):
    from concourse.bass_utils import run_bass_kernel_spmd

    if "nc" not in _cache:
        _cache["nc"] = _build_program()
    nc = _cache["nc"]

    xep, xop, K, sel, xsT = _prep_shared(x)
    selmat = (np.arange(OS) // SHARE == np.arange(OGS)[:, None]).astype(np.float32)

    in_maps = []
    for m in range(NCORES):
        wq = np.ascontiguousarray(weight_q4[m * OS:(m + 1) * OS]).astype('<i4')
        wq16 = wq.view('<u2').reshape(OS, I)

        norm = weight_norm[m * OGS:(m + 1) * OGS, :, 0].astype(np.float32)  # [64, 256]
        sn = (2.0 / 15.0) * norm
        # s2[m, pr, og] = sn[og, 8*pr + m//16]
        blk = 8 * np.arange(NPAIR)[None, :] + (np.arange(128) // 16)[:, None]
        s2 = sn.T[blk].astype(np.float16)                 # [128, 32, 64]

        nmT = np.ascontiguousarray(
            norm.T.reshape(2, 128, OGS).transpose(1, 0, 2)).astype(np.float32)

        rhs_fix = np.empty((OGS + 1, OS), np.float32)
        rhs_fix[0:OGS] = selmat
        rhs_fix[OGS] = bias[m * OS:(m + 1) * OS].astype(np.float32)

        in_maps.append(dict(
            wq16=wq16, xepat=xep.view(np.uint16), xopat=xop.view(np.uint16),
            s2=s2.view(np.uint16), koff=K, sel=sel.view(np.uint16),
            xsT=xsT, normT=nmT, rhs_fix=rhs_fix))

    res = run_bass_kernel_spmd(nc, in_maps, core_ids=list(range(NCORES)),
                               trace=_trace, **(_trace_kwargs or {}))
    outs = [r["out"] for r in res.results]
    full = np.concatenate(outs, axis=1).astype(np.float32)
    if _trace:
        return full, res
    return full


# revision 16
# speedup vs baseline: 1.6047x; 1.0251x over previous
"""Trainium2 kernel for nn_LinearRowShared4Bit: out = x @ W.T + bias where W is
dequantized from 4-bit packed weights with per-(16-row-group, 32-col-block)
fp16 norms.

8-core tensor-parallel over out_features (1024 rows/core). Per core:

  - View int32 packed weights (value = byte B in low 8 bits) as uint16 pairs
    [B, 0]; DMA-transpose 128-column slices into SBUF -> T tiles [128p, o]
    with B on even partitions, 0 on odd. Partition p=2m of chunk c holds byte
    k=64c+m -> (b,h)=(k//16,k%16) -> W.T rows i_lo=32b+2h (low nib), i_lo+1
    (high nib).
  - Bit-assemble fp16 weights with pure-bitwise DVE ops (no int->fp convert):
      F_L = (T & 15) | 0x6400   == fp16(1024 + L)
      F_T =  T       | 0x6400   == fp16(1024 + T),  T = 16H + L
  - Stage 1 (PE): per chunk-pair, accumulate per-block-slot partials into
    PSUM [128=(16t x 8slot), 1024o]:  ps[16j+t, o] += sum_p xepat*F_L
    + xopat*F_T, where xepat/xopat are host-prepped block-masked x patterns
    (xe - xo/16 and xo/16) so that L/H separate algebraically:
      sum_i x*s*q = sum (xe - xo/16)*(s*L) + sum (xo/16)*(s*T).
  - Stage 2: one fused DVE op per pair: (ps - K[m]) * s_bcast, where K[m] is
    the host-computed 1024-offset contribution, s[m, og(o)] the norm scale;
    then a selector matmul reduces the 8 block-slots into PSUM out [16, 1024].
  - The "-norm" dequant term and bias ride a small fp32 side-matmul.

Host gathers per-core [16, 1024] outputs -> [16, 8192].
"""

import numpy as np

O, I = 8192, 8192
GROUP, SHARE = 32, 16
NCORES = 8
OS = O // NCORES          # 1024 out rows per core
OGS = OS // SHARE         # 64 row-groups per core
NCHUNK = I // 128         # 64 col-chunks of 128 uint16 columns
NPAIR = NCHUNK // 2
T_BATCH = 16

_cache = {}


def _build_program():
    import concourse.mybir as mybir
    from concourse import bacc
    from concourse.tile import TileContext

    from concourse.tile_rust import add_dep_helper

    f16, f32, u16 = mybir.dt.float16, mybir.dt.float32, mybir.dt.uint16
    alu = mybir.AluOpType
    nc = bacc.Bacc("TRN2", target_bir_lowering=False, debug=False)

    wq16 = nc.dram_tensor("wq16", [OS, I], u16, kind="ExternalInput")
    xep_d = nc.dram_tensor("xepat", [128, NCHUNK, 64], f16, kind="ExternalInput")
    xop_d = nc.dram_tensor("xopat", [128, NCHUNK, 64], f16, kind="ExternalInput")
    s2_d = nc.dram_tensor("s2", [128, NPAIR, OGS], f16, kind="ExternalInput")
    k_d = nc.dram_tensor("koff", [128, NPAIR], f32, kind="ExternalInput")
    sel_d = nc.dram_tensor("sel", [128, T_BATCH], f16, kind="ExternalInput")
    xsT_d = nc.dram_tensor("xsT", [128, 2, T_BATCH], f32, kind="ExternalInput")
    nmT_d = nc.dram_tensor("normT", [128, 2, OGS], f32, kind="ExternalInput")
    fx_d = nc.dram_tensor("rhs_fix", [OGS + 1, OS], f32, kind="ExternalInput")
    out_d = nc.dram_tensor("out", [T_BATCH, OS], f32, kind="ExternalOutput")

    with TileContext(nc) as tc:
        with (
            tc.tile_pool(name="const", bufs=1) as const,
            tc.tile_pool(name="tp", bufs=6) as tp,
            tc.tile_pool(name="wp", bufs=3) as wp,
            tc.tile_pool(name="ps", bufs=1, space="PSUM") as ps,
        ):
            load_insts = []
            xep_sb = const.tile([128, NCHUNK, 64], f16)
            load_insts.append(nc.sync.dma_start(xep_sb[:], xep_d[:]))
            xop_sb = const.tile([128, NCHUNK, 64], f16)
            load_insts.append(nc.sync.dma_start(xop_sb[:], xop_d[:]))
            s2_sb = const.tile([128, NPAIR, OGS], f16)
            load_insts.append(nc.sync.dma_start(s2_sb[:], s2_d[:]))
            k_sb = const.tile([128, NPAIR], f32)
            load_insts.append(nc.sync.dma_start(k_sb[:], k_d[:]))
            sel_sb = const.tile([128, T_BATCH], f16)
            load_insts.append(nc.sync.dma_start(sel_sb[:], sel_d[:]))
            xsT_sb = const.tile([128, 2, T_BATCH], f32)
            load_insts.append(nc.sync.dma_start(xsT_sb[:], xsT_d[:]))
            nmT_sb = const.tile([128, 2, OGS], f32)
            load_insts.append(nc.sync.dma_start(nmT_sb[:], nmT_d[:]))
            fx_sb = const.tile([OGS + 1, OS], f32)
            load_insts.append(nc.sync.dma_start(fx_sb[:], fx_d[:]))
            mask = const.tile([128, 1], u16)
            nc.vector.memset(mask[:], 15)
            orc = const.tile([128, 1], u16)
            nc.vector.memset(orc[:], 0x6400)

            psA = ps.tile([T_BATCH, 512], f32)
            psB = ps.tile([T_BATCH, 512], f32)

            # fix path: N.T[og, t] = sum_b norm[og, b] * xs[t, b]
            ps2 = ps.tile([OGS, T_BATCH], f32, tag="pp", bufs=3)
            nc.tensor.matmul(ps2[:], nmT_sb[:, 0, :], xsT_sb[:, 0, :],
                             start=True, stop=False)
            nc.tensor.matmul(ps2[:], nmT_sb[:, 1, :], xsT_sb[:, 1, :],
                             start=False, stop=True)
            fixw = const.tile([OGS + 1, T_BATCH], f32)
            nc.vector.tensor_scalar_mul(fixw[0:OGS, :], ps2[:], -1.0)
            nc.vector.memset(fixw[OGS:OGS + 1, :], 1.0)

            for pr in range(NPAIR):
                T2 = tp.tile([128, 2048], u16, tag="T")
                d0 = d1 = nc.sync
                t0 = d0.dma_start_transpose(T2[:, 0:1024],
                                            wq16[:, 256 * pr:256 * pr + 128])
                d1.dma_start_transpose(T2[:, 1024:2048],
                                       wq16[:, 256 * pr + 128:256 * pr + 256])
                if pr == 1:
                    # the scalar HWDGE ring's first transpose must not run
                    # while plain loads are in flight (xbar-mode corruption)
                    for li in load_insts:
                        add_dep_helper(t0.ins, li.ins, sync=True,
                                       reason="xbar: transpose after plain DMAs")

                FL = wp.tile([128, 2048], u16, tag="FL", bufs=6)
                nc.vector.tensor_scalar(FL[:], T2[:], mask[:], orc[:],
                                        alu.bitwise_and, alu.bitwise_or)
                FT = wp.tile([128, 2048], u16, tag="FT", bufs=6)
                nc.vector.tensor_scalar(FT[:], T2[:], orc[:], None,
                                        alu.bitwise_or)
                FLh = FL[:].bitcast(f16)
                FTh = FT[:].bitcast(f16)

                pp = ps.tile([128, 1024], f32, tag="pp", bufs=3)
                for h in (0, 1):
                    c = 2 * pr + h
                    xe_l = xep_sb[:, c, :]
                    xo_l = xop_sb[:, c, :]
                    o0 = 1024 * h
                    rows = pp[64 * h:64 * h + 64, :]
                    nc.tensor.matmul(rows[:, 0:512], xe_l,
                                     FLh[:, o0:o0 + 512],
                                     start=True, stop=False)
                    nc.tensor.matmul(rows[:, 512:1024], xe_l,
                                     FLh[:, o0 + 512:o0 + 1024],
                                     start=True, stop=False)
                    nc.tensor.matmul(rows[:, 0:512], xo_l,
                                     FTh[:, o0:o0 + 512],
                                     start=False, stop=True)
                    nc.tensor.matmul(rows[:, 512:1024], xo_l,
                                     FTh[:, o0 + 512:o0 + 1024],
                                     start=False, stop=True)

                sc = wp.tile([128, 1024], f16, tag="SC", bufs=6)
                nc.vector.scalar_tensor_tensor(
                    sc[:].rearrange("p (a b) -> p a b", b=SHARE),
                    pp[:].rearrange("p (a b) -> p a b", b=SHARE),
                    k_sb[:, pr:pr + 1],
                    s2_sb[:, pr, :].unsqueeze(2).broadcast_to([128, OGS, SHARE]),
                    alu.subtract, alu.mult)

                nc.tensor.matmul(psA[:], sel_sb[:], sc[:, 0:512],
                                 start=(pr == 0), stop=False)
                nc.tensor.matmul(psB[:], sel_sb[:], sc[:, 512:1024],
                                 start=(pr == 0), stop=False)

            nc.tensor.matmul(psA[:], fixw[:], fx_sb[:, 0:512],
                             start=False, stop=True)
            nc.tensor.matmul(psB[:], fixw[:], fx_sb[:, 512:1024],
                             start=False, stop=True)
            out_sb = const.tile([T_BATCH, OS], f32)
            nc.vector.tensor_copy(out_sb[:, 0:512], psA[:])
            nc.vector.tensor_copy(out_sb[:, 512:1024], psB[:])
            nc.sync.dma_start(out_d[:], out_sb[:])

    nc.finalize()
    return nc


def _prep_shared(x):
    """x-derived operands, identical on every core."""
    xf = x.astype(np.float64)
    k = np.arange(I // 2)                   # byte index within a row
    b, h = k // 16, k % 16
    i_lo = 32 * b + 2 * h
    xe_mod = xf[:, i_lo] - xf[:, i_lo + 1] / 16.0   # [16, 4096]
    xo16 = xf[:, i_lo + 1] / 16.0                    # [16, 4096]

    def pat(a):
        """[16, 4096] -> [128, 64, 64] fp16 block-slot pattern.

        Payload lane p=2m of chunk c holds byte k=64c+m; its x value goes to
        column m' = 16*(p//32) + t. Odd lanes and other columns stay 0."""
        lane = np.zeros((128, NCHUNK, T_BATCH), np.float16)
        lane[0::2] = a.T.reshape(NCHUNK, 64, T_BATCH).transpose(1, 0, 2)
        out = np.zeros((128, NCHUNK, 64), np.float16)
        for jj in range(4):
            rows = slice(32 * jj, 32 * jj + 32)
            out[rows, :, 16 * jj:16 * jj + 16] = lane[rows]
        return out

    xep = pat(xe_mod)
    xop = pat(xo16)

    # K[m, pr] = 1024 * sum_p (xep + xop)[p, c, m%64] with c = 2pr + m//64,
    # computed from the fp16-rounded patterns (must match device exactly).
    colsum = (xep.astype(np.float64) + xop.astype(np.float64)).sum(axis=0)  # [64c, 64m]
    K = np.zeros((128, NPAIR), np.float32)
    K[0:64] = 1024.0 * colsum[0::2].T
    K[64:128] = 1024.0 * colsum[1::2].T

    sel = (np.arange(128)[:, None] % 16 == np.arange(T_BATCH)[None, :]
           ).astype(np.float16)

    xs = xf.reshape(T_BATCH, I // GROUP, GROUP).sum(-1)   # [16, 256]
    xsT = np.ascontiguousarray(
        xs.T.reshape(2, 128, T_BATCH).transpose(1, 0, 2)).astype(np.float32)
    return xep, xop, K, sel, xsT


def kernel(x, weight_q4, weight_norm, bias, _trace=False, _trace_kwargs=None):
    from concourse.bass_utils import run_bass_kernel_spmd

    if "nc" not in _cache:
        _cache["nc"] = _build_program()
    nc = _cache["nc"]

    xep, xop, K, sel, xsT = _prep_shared(x)
    selmat = (np.arange(OS) // SHARE == np.arange(OGS)[:, None]).astype(np.float32)

    in_maps = []
    for m in range(NCORES):
        wq = np.ascontiguousarray(weight_q4[m * OS:(m + 1) * OS]).astype('<i4')
        wq16 = wq.view('<u2').reshape(OS, I)

        norm = weight_norm[m * OGS:(m + 1) * OGS, :, 0].astype(np.float32)  # [64, 256]
        sn = (2.0 / 15.0) * norm
        # s2[m, pr, og] = sn[og, 8*pr + m//16]
        blk = 8 * np.arange(NPAIR)[None, :] + (np.arange(128) // 16)[:, None]
        s2 = sn.T[blk].astype(np.float16)                 # [128, 32, 64]

        nmT = np.ascontiguousarray(
            norm.T.reshape(2, 128, OGS).transpose(1, 0, 2)).astype(np.float32)

        rhs_fix = np.empty((OGS + 1, OS), np.float32)
        rhs_fix[0:OGS] = selmat
        rhs_fix[OGS] = bias[m * OS:(m + 1) * OS].astype(np.float32)

        in_maps.append(dict(
            wq16=wq16, xepat=xep.view(np.uint16), xopat=xop.view(np.uint16),
            s2=s2.view(np.uint16), koff=K, sel=sel.view(np.uint16),
            xsT=xsT, normT=nmT, rhs_fix=rhs_fix))

    res = run_bass_kernel_spmd(nc, in_maps, core_ids=list(range(NCORES)),
                               trace=_trace, **(_trace_kwargs or {}))
    outs = [r["out"] for r in res.results]
    full = np.concatenate(outs, axis=1).astype(np.float32)
    if _trace:
        return full, res
    return full


# revision 17
# speedup vs baseline: 1.6277x; 1.0144x over previous
"""Trainium2 kernel for nn_LinearRowShared4Bit: out = x @ W.T + bias where W is
dequantized from 4-bit packed weights with per-(16-row-group, 32-col-block)
fp16 norms.

8-core tensor-parallel over out_features (1024 rows/core). Per core:

  - View int32 packed weights (value = byte B in low 8 bits) as uint16 pairs
    [B, 0]; DMA-transpose quarter-shards [1024 o, 2048 cols] -> SBUF
    [128p, 16 chunk, 1024 o] (4KB-contiguous M2S reads -> ~278 GB/s). Byte
    k=64c+m of chunk c sits on partition p=2m -> (b,h)=(k//16,k%16) ->
    W.T rows i_lo=32b+2h (low nibble), i_lo+1 (high nibble); odd p are 0.
    All DMAs ride ONE HWDGE ring (nc.sync): concurrent plain DMAs corrupt
    in-flight xbar transposes (engine-global S2M xbar state).
  - Bit-assemble fp16 weights with pure-bitwise DVE ops (no int->fp convert):
      F_L = (T & 15) | 0x6400   == fp16(1024 + L)
      F_T =  T       | 0x6400   == fp16(1024 + T),  T = 16H + L
  - Stage 1 (PE): per chunk-pair, accumulate per-block-slot partials into
    PSUM [128=(16t x 8slot), 1024o] with host-prepped block-masked x patterns
    xepat (xe - xo/16) against F_L and xopat (xo/16) against F_T, so nibbles
    separate algebraically: sum x*s*q = sum(xe-xo/16)(s*L) + sum(xo/16)(s*T).
  - Stage 2: one fused DVE scalar_tensor_tensor per pair:
    (psum - K[m]) * s[m, og(o)], K = host-computed 1024-offset contribution;
    a selector matmul folds the 8 block-slots into PSUM out [16, 1024].
  - The "-norm" dequant term and bias ride a small fp32 side-matmul.

Host gathers per-core [16, 1024] outputs -> [16, 8192].
"""

import numpy as np

O, I = 8192, 8192
GROUP, SHARE = 32, 16
NCORES = 8
OS = O // NCORES          # 1024 out rows per core
OGS = OS // SHARE         # 64 row-groups per core
NCHUNK = I // 128         # 64 col-chunks of 128 uint16 columns
NPAIR = NCHUNK // 2
NQ = 4                    # quarter transposes, 16 chunks each
T_BATCH = 16

# packed f16 const layout (u16 columns): xep | xop | s2 | sel
_XEP0, _XOP0 = 0, NCHUNK * 64
_S20 = 2 * NCHUNK * 64
_SEL0 = _S20 + NPAIR * OGS
_C16W = _SEL0 + T_BATCH
# packed f32 const layout: koff | xsT | normT
_K0, _XST0, _NMT0 = 0, NPAIR, NPAIR + 2 * T_BATCH
_C32W = _NMT0 + 2 * OGS

_cache = {}


def _build_program():
    import concourse.mybir as mybir
    from concourse import bacc
    from concourse.tile import TileContext

    f16, f32, u16 = mybir.dt.float16, mybir.dt.float32, mybir.dt.uint16
    alu = mybir.AluOpType
    nc = bacc.Bacc("TRN2", target_bir_lowering=False, debug=False)

    wq16 = nc.dram_tensor("wq16", [OS, I], u16, kind="ExternalInput")
    c16_d = nc.dram_tensor("c16", [128, _C16W], u16, kind="ExternalInput")
    c32_d = nc.dram_tensor("c32", [128, _C32W], f32, kind="ExternalInput")
    fx_d = nc.dram_tensor("rhs_fix", [OGS + 1, OS], f32, kind="ExternalInput")
    out_d = nc.dram_tensor("out", [T_BATCH, OS], f32, kind="ExternalOutput")

    with TileContext(nc) as tc:
        with (
            tc.tile_pool(name="const", bufs=1) as const,
            tc.tile_pool(name="tp", bufs=2) as tp,
            tc.tile_pool(name="wp", bufs=3) as wp,
            tc.tile_pool(name="ps", bufs=1, space="PSUM") as ps,
        ):
            c16 = const.tile([128, _C16W], u16)
            nc.sync.dma_start(c16[:], c16_d[:])
            c32 = const.tile([128, _C32W], f32)
            nc.sync.dma_start(c32[:], c32_d[:])
            fx_sb = const.tile([OGS + 1, OS], f32)
            nc.sync.dma_start(fx_sb[:], fx_d[:])
            mask = const.tile([128, 1], u16)
            nc.vector.memset(mask[:], 15)
            orc = const.tile([128, 1], u16)
            nc.vector.memset(orc[:], 0x6400)

            xep_sb = c16[:, _XEP0:_XOP0].rearrange(
                "p (c m) -> p c m", m=64).bitcast(f16)
            xop_sb = c16[:, _XOP0:_S20].rearrange(
                "p (c m) -> p c m", m=64).bitcast(f16)
            s2_sb = c16[:, _S20:_SEL0].rearrange(
                "p (r g) -> p r g", g=OGS).bitcast(f16)
            sel_sb = c16[:, _SEL0:_C16W].bitcast(f16)
            k_sb = c32[:, _K0:_XST0]
            xsT_sb = c32[:, _XST0:_NMT0].rearrange("p (j t) -> p j t", t=T_BATCH)
            nmT_sb = c32[:, _NMT0:_C32W].rearrange("p (j g) -> p j g", g=OGS)

            psA = ps.tile([T_BATCH, 512], f32)
            psB = ps.tile([T_BATCH, 512], f32)

            # fix path: N.T[og, t] = sum_b norm[og, b] * xs[t, b]
            ps2 = ps.tile([OGS, T_BATCH], f32, tag="pp", bufs=3)
            nc.tensor.matmul(ps2[:], nmT_sb[:, 0, :], xsT_sb[:, 0, :],
                             start=True, stop=False)
            nc.tensor.matmul(ps2[:], nmT_sb[:, 1, :], xsT_sb[:, 1, :],
                             start=False, stop=True)
            fixw = const.tile([OGS + 1, T_BATCH], f32)
            nc.vector.tensor_scalar_mul(fixw[0:OGS, :], ps2[:], -1.0)
            nc.vector.memset(fixw[OGS:OGS + 1, :], 1.0)

            for q in range(NQ):
                TQ = tp.tile([128, 16, 1024], u16, tag="T", name=f"tq{q}")
                nc.sync.dma_start_transpose(
                    TQ[:], wq16[:, 2048 * q:2048 * (q + 1)])

                for pl in range(8):
                    pr = 8 * q + pl
                    T2 = TQ[:, 2 * pl:2 * pl + 2, :]

                    FL = wp.tile([128, 2048], u16, tag="FL")
                    nc.vector.tensor_scalar(
                        FL[:].rearrange("p (a b) -> p a b", b=1024), T2,
                        mask[:], orc[:], alu.bitwise_and, alu.bitwise_or)
                    FT = wp.tile([128, 2048], u16, tag="FT")
                    nc.vector.tensor_scalar(
                        FT[:].rearrange("p (a b) -> p a b", b=1024), T2,
                        orc[:], None, alu.bitwise_or)
                    FLh = FL[:].bitcast(f16)
                    FTh = FT[:].bitcast(f16)

                    pp = ps.tile([128, 1024], f32, tag="pp", bufs=3)
                    for h in (0, 1):
                        c = 2 * pr + h
                        xe_l = xep_sb[:, c, :]
                        xo_l = xop_sb[:, c, :]
                        o0 = 1024 * h
                        rows = pp[64 * h:64 * h + 64, :]
                        nc.tensor.matmul(rows[:, 0:512], xe_l,
                                         FLh[:, o0:o0 + 512],
                                         start=True, stop=False)
                        nc.tensor.matmul(rows[:, 512:1024], xe_l,
                                         FLh[:, o0 + 512:o0 + 1024],
                                         start=True, stop=False)
                        nc.tensor.matmul(rows[:, 0:512], xo_l,
                                         FTh[:, o0:o0 + 512],
                                         start=False, stop=True)
                        nc.tensor.matmul(rows[:, 512:1024], xo_l,
                                         FTh[:, o0 + 512:o0 + 1024],
                                         start=False, stop=True)

                    sc = wp.tile([128, 1024], f16, tag="SC")
                    nc.vector.scalar_tensor_tensor(
                        sc[:].rearrange("p (a b) -> p a b", b=SHARE),
                        pp[:].rearrange("p (a b) -> p a b", b=SHARE),
                        k_sb[:, pr:pr + 1],
                        s2_sb[:, pr, :].unsqueeze(2).broadcast_to(
                            [128, OGS, SHARE]),
                        alu.subtract, alu.mult)

                    nc.tensor.matmul(psA[:], sel_sb[:], sc[:, 0:512],
                                     start=(pr == 0), stop=False)
                    nc.tensor.matmul(psB[:], sel_sb[:], sc[:, 512:1024],
                                     start=(pr == 0), stop=False)

            nc.tensor.matmul(psA[:], fixw[:], fx_sb[:, 0:512],
                             start=False, stop=True)
            nc.tensor.matmul(psB[:], fixw[:], fx_sb[:, 512:1024],
                             start=False, stop=True)
            out_sb = const.tile([T_BATCH, OS], f32)
            nc.vector.tensor_copy(out_sb[:, 0:512], psA[:])
            nc.vector.tensor_copy(out_sb[:, 512:1024], psB[:])
            nc.sync.dma_start(out_d[:], out_sb[:])

    nc.finalize()
    return nc


def _prep_shared(x):
    """x-derived operands, identical on every core."""
    xf = x.astype(np.float64)
    k = np.arange(I // 2)                   # byte index within a row
    b, h = k // 16, k % 16
    i_lo = 32 * b + 2 * h
    xe_mod = xf[:, i_lo] - xf[:, i_lo + 1] / 16.0   # [16, 4096]
    xo16 = xf[:, i_lo + 1] / 16.0                    # [16, 4096]

    def pat(a):
        """[16, 4096] -> [128, 64, 64] fp16 block-slot pattern.

        Payload lane p=2m of chunk c holds byte k=64c+m; its x value goes to
        column m' = 16*(p//32) + t. Odd lanes and other columns stay 0."""
        lane = np.zeros((128, NCHUNK, T_BATCH), np.float16)
        lane[0::2] = a.T.reshape(NCHUNK, 64, T_BATCH).transpose(1, 0, 2)
        out = np.zeros((128, NCHUNK, 64), np.float16)
        for jj in range(4):
            rows = slice(32 * jj, 32 * jj + 32)
            out[rows, :, 16 * jj:16 * jj + 16] = lane[rows]
        return out

    xep = pat(xe_mod)
    xop = pat(xo16)

    # K[m, pr] = 1024 * sum_p (xep + xop)[p, c, m%64] with c = 2pr + m//64,
    # computed from the fp16-rounded patterns (must match device exactly).
    colsum = (xep.astype(np.float64) + xop.astype(np.float64)).sum(axis=0)
    K = np.zeros((128, NPAIR), np.float32)
    K[0:64] = 1024.0 * colsum[0::2].T
    K[64:128] = 1024.0 * colsum[1::2].T

    sel = (np.arange(128)[:, None] % 16 == np.arange(T_BATCH)[None, :]
           ).astype(np.float16)

    xs = xf.reshape(T_BATCH, I // GROUP, GROUP).sum(-1)   # [16, 256]
    xsT = np.ascontiguousarray(
        xs.T.reshape(2, 128, T_BATCH).transpose(1, 0, 2)).astype(np.float32)
    return xep, xop, K, sel, xsT


def kernel(x, weight_q4, weight_norm, bias, _trace=False, _trace_kwargs=None):
    from concourse.bass_utils import run_bass_kernel_spmd

    if "nc" not in _cache:
        _cache["nc"] = _build_program()
    nc = _cache["nc"]

    xep, xop, K, sel, xsT = _prep_shared(x)
    selmat = (np.arange(OS) // SHARE == np.arange(OGS)[:, None]).astype(np.float32)

    c16 = np.empty((128, _C16W), np.uint16)
    c16[:, _XEP0:_XOP0] = xep.reshape(128, -1).view(np.uint16)
    c16[:, _XOP0:_S20] = xop.reshape(128, -1).view(np.uint16)
    c16[:, _SEL0:_C16W] = sel.view(np.uint16)

    c32 = np.empty((128, _C32W), np.float32)
    c32[:, _K0:_XST0] = K
    c32[:, _XST0:_NMT0] = xsT.reshape(128, -1)

    in_maps = []
    for m in range(NCORES):
        wq = np.ascontiguousarray(weight_q4[m * OS:(m + 1) * OS]).astype('<i4')
        wq16 = wq.view('<u2').reshape(OS, I)

        norm = weight_norm[m * OGS:(m + 1) * OGS, :, 0].astype(np.float32)
        sn = (2.0 / 15.0) * norm
        # s2[m, pr, og] = sn[og, 8*pr + m//16]
        blk = 8 * np.arange(NPAIR)[None, :] + (np.arange(128) // 16)[:, None]
        s2 = sn.T[blk].astype(np.float16)                 # [128, 32, 64]

        nmT = np.ascontiguousarray(
            norm.T.reshape(2, 128, OGS).transpose(1, 0, 2)).astype(np.float32)

        c16m = c16.copy()
        c16m[:, _S20:_SEL0] = s2.reshape(128, -1).view(np.uint16)
        c32m = c32.copy()
        c32m[:, _NMT0:_C32W] = nmT.reshape(128, -1)

        rhs_fix = np.empty((OGS + 1, OS), np.float32)
        rhs_fix[0:OGS] = selmat
        rhs_fix[OGS] = bias[m * OS:(m + 1) * OS].astype(np.float32)

        in_maps.append(dict(wq16=wq16, c16=c16m, c32=c32m, rhs_fix=rhs_fix))

    res = run_bass_kernel_spmd(nc, in_maps, core_ids=list(range(NCORES)),
                               trace=_trace, **(_trace_kwargs or {}))
    outs = [r["out"] for r in res.results]
    full = np.concatenate(outs, axis=1).astype(np.float32)
    if _trace:
        return full, res
    return full
